# revision 17
# baseline (speedup 1.0000x reference)
"""Cross-attention block kernel for Trainium2 (8 NeuronCores, data-parallel over batch).

Reference computation (per batch element b):
    Q = q[b] @ Wq; K = k[b] @ Wk; V = v[b] @ Wv        # [4096, 128] each
    O = softmax(Q @ K^T / sqrt(128)) @ V               # [4096, 128]

Sharding: one batch element per core (B == n_cores == 8), weights replicated.

Per-core device algorithm (layouts chosen so the PE contracts along
partitions with zero on-device transposes of the big activations):
  - Host supplies q/k/v pre-transposed per core: qT/kT/vT = [1024, 4096]
    (feature-major) - a pure layout transformation done while sharding.
  - Projections (fp32r, moving N=256): QT[AD,q], KT[AD,kv], VT[AD,kv].
  - VT is PE-transposed tile-wise into V natural [kv, AD] (fp16) with an
    appended ones-column (V_aug[kv, 129]) -> the PV matmul then yields both
    the unnormalized output and the softmax denominator in one pass.
  - Scores TRANSPOSED:  ST[kv, q] = lhsT(KT tile).T @ QT chunk   (fp32r)
  - E = exp(ST * scale) on the scalar engine (PSUM->SBUF, fp16 out),
    batched 2 kv-tiles (1024 wide) per activation to amortize the fixed
    ~352-cycle ACTIVATE overhead.  No max-subtraction: scores are O(1)
    for randn-scaled inputs, exp is safe in fp32.  E[kv, q] tiles are
    exactly the stationary operand for the PV matmul.
  - O_aug[q, 129] = sum_kv E_tile.T @ V_aug_tile  (fp16 operands, fp32 PSUM)
    O = O_aug[:, :128] * (1 / O_aug[:, 128]) per row, fp32 out.

Emission is driven by a greedy software-pipeliner that interleaves three
streams under resource bookkeeping so DMA (~134 us of input), PE (~155 us
of matmul) and ACT (~147 us of exp) overlap end to end:
  - spine: per-chunk input DMA + projection units (k/q/v), demand-pulled
    when a downstream stream blocks on them and otherwise paced vs scores;
  - scores: st2(c, kc) units (2 matmuls + 1 exp batched over 2 kv tiles),
    gated on kt/qt and on one of 7 E-half SBUF slots;
  - pv: strict-order quarter units (2 q-tiles x 4 kv tiles) accumulating
    into 2 PSUM chains, gated on vn chunks and E completeness; emitting pv
    quarters is also forced whenever scores need an E slot freed (keeps the
    in-order PE stream deadlock-free w.r.t. the sp ping-pong with ACT).
"""

import os
import sys

import numpy as np

for _p in ("/opt/trn_rl_repo",):
    if _p not in sys.path and os.path.isdir(_p):
        sys.path.insert(0, _p)

import concourse.bacc as bacc
import concourse.tile as tile
from concourse import mybir
from concourse.bass_utils import run_bass_kernel_spmd
from concourse.masks import make_identity

F32 = mybir.dt.float32
F32R = mybir.dt.float32r
BF16 = mybir.dt.bfloat16
FP16 = mybir.dt.float16

B, NQ, NKV, QD, KVD, AD = 8, 4096, 4096, 1024, 1024, 128
P = 128
FT = QD // P          # 8 feature tiles (projection contraction tiles)
KVT = NKV // P        # 32 kv tiles
QCHUNK = 512
NQC = NQ // QCHUNK    # 8 q chunks
TPC = QCHUNK // P     # 4 kv/q tiles per chunk
QKV = KVT // 4        # kv tiles per E-quarter (8)
VW = 132              # V_aug tile width (128 vals + ones col + pad)
XH = 256              # input-stream half-chunk width
SCALE = float(AD) ** -0.5
E_SLOTS = 12
OP_SLOTS = 2

TRACE = False         # test.py flips this for profiling runs
_TRACE_KW = {}


def build_bass():
    nc = bacc.Bacc("TRN2", target_bir_lowering=False, debug=False, num_devices=B)

    qT = nc.dram_tensor("qT", [QD, NQ], F32, kind="ExternalInput")
    kT = nc.dram_tensor("kT", [KVD, NKV], F32, kind="ExternalInput")
    vT = nc.dram_tensor("vT", [KVD, NKV], F32, kind="ExternalInput")
    wq = nc.dram_tensor("wq", [QD, AD], F32, kind="ExternalInput")
    wk = nc.dram_tensor("wk", [KVD, AD], F32, kind="ExternalInput")
    wv = nc.dram_tensor("wv", [KVD, AD], F32, kind="ExternalInput")
    out = nc.dram_tensor("out", [NQ, AD], F32, kind="ExternalOutput")

    with tile.TileContext(nc) as tc:
        with (
            tc.tile_pool(name="const", bufs=1) as const,
            tc.tile_pool(name="persist", bufs=1) as persist,
            tc.tile_pool(name="xin", bufs=2) as xin,
            tc.tile_pool(name="work", bufs=2) as work,
            tc.tile_pool(name="epool", bufs=E_SLOTS) as epool,
            tc.tile_pool(name="ps_sp", bufs=3, space="PSUM") as ps_sp,
            tc.tile_pool(name="ps_pl", bufs=2, space="PSUM") as ps_pl,
        ):
            # ---- weights / identity
            w_sb = {}
            for name, w in (("q", wq), ("k", wk), ("v", wv)):
                t = const.tile([P, FT, AD], F32R, tag=f"w{name}", name=f"w{name}")
                nc.sync.dma_start(
                    out=t, in_=w.ap().rearrange("(t p) a -> p t a", p=P).bitcast(F32R)
                )
                w_sb[name] = t
            ident = const.tile([P, P], F32)
            make_identity(nc, ident)

            # ---- per-chunk persistent tiles (separate tags => chunk-granular deps)
            qt_t = [persist.tile([P, QCHUNK], F32R, tag=f"qt{c}", name=f"qt{c}")
                    for c in range(NQC)]
            kt_t = [persist.tile([P, QCHUNK], F32R, tag=f"kt{c}", name=f"kt{c}")
                    for c in range(NQC)]
            vn_t = [persist.tile([P, TPC, VW], FP16, tag=f"vn{c}", name=f"vn{c}")
                    for c in range(NQC)]
            oacc = [persist.tile([P, TPC, VW], F32, tag=f"oacc{c}", name=f"oacc{c}")
                    for c in range(NQC)]

            def proj_half(src_dram, which, c, h):
                """DMA + project one 256-wide half of chunk c; returns psum ap."""
                src = xin.tile([P, FT, XH], F32R, tag="xin", name="xin")
                ap = (src_dram.ap()[:, c * QCHUNK + h * XH: c * QCHUNK + (h + 1) * XH]
                      .rearrange("(t p) n -> p t n", p=P).bitcast(F32R))
                nc.sync.dma_start(out=src, in_=ap)
                pp = ps_sp.tile([P, 2 * QCHUNK], F32, tag="sp", name="pp")
                for t in range(FT):
                    nc.tensor.matmul(
                        pp[:, 0:XH], w_sb[which][:, t, :], src[:, t, :],
                        start=(t == 0), stop=(t == FT - 1),
                    )
                return pp

            def k_chunk(c):
                for h in range(2):
                    pp = proj_half(kT, "k", c, h)
                    nc.vector.tensor_copy(
                        kt_t[c][:, h * XH:(h + 1) * XH], pp[:, 0:XH])

            def q_chunk(c):
                for h in range(2):
                    pp = proj_half(qT, "q", c, h)
                    nc.vector.tensor_copy(
                        qt_t[c][:, h * XH:(h + 1) * XH], pp[:, 0:XH])

            def v_chunk(c):
                vt_sb = work.tile([P, QCHUNK], F32, tag="vt", name="vt")
                for h in range(2):
                    pp = proj_half(vT, "v", c, h)
                    nc.vector.tensor_copy(vt_sb[:, h * XH:(h + 1) * XH], pp[:, 0:XH])
                nc.vector.memset(vn_t[c], 1.0)
                for j in range(TPC):
                    tp = ps_sp.tile([P, 2 * QCHUNK], F32, tag="sp", name="tp")
                    nc.tensor.transpose(tp[:, 0:P], vt_sb[:, j * P:(j + 1) * P], ident)
                    nc.vector.tensor_copy(vn_t[c][:, j, 0:P], tp[:, 0:P])

            # ---- stream state -------------------------------------------------
            spine = [
                ("k", 0), ("q", 0), ("q", 1), ("k", 1), ("q", 2),
                ("k", 2), ("k", 3), ("k", 4), ("k", 5), ("k", 6), ("k", 7),
                ("q", 3), ("v", 0), ("v", 1), ("q", 4), ("v", 2), ("v", 3),
                ("q", 5), ("v", 4), ("v", 5), ("q", 6), ("v", 6), ("v", 7),
                ("q", 7),
            ]
            spine_pos = 0
            kt_done = [False] * NQC
            qt_done = [False] * NQC
            vn_done = [False] * NQC

            def emit_spine():
                nonlocal spine_pos
                kind, c = spine[spine_pos]
                spine_pos += 1
                if kind == "k":
                    k_chunk(c)
                    kt_done[c] = True
                elif kind == "q":
                    q_chunk(c)
                    qt_done[c] = True
                else:
                    v_chunk(c)
                    vn_done[c] = True

            def emit_spine_where(pred):
                while spine_pos < len(spine):
                    if pred(spine[spine_pos]):
                        emit_spine()
                        return True
                    emit_spine()
                return False

            # E-quarter slots
            E_tiles = {}
            e_alive = set()

            def E_of(c, qq):
                key = (c, qq)
                if key not in E_tiles:
                    E_tiles[key] = epool.tile(
                        [P, QKV, QCHUNK], FP16, tag="E", name=f"E{c}_{qq}"
                    )
                    e_alive.add(key)
                return E_tiles[key]

            # scores stream: first 3 chunks ride the k stream, rest c-major
            st_units = (
                [(c, kc) for kc in range(NQC) for c in range(3)]
                + [(c, kc) for c in range(3, NQC) for kc in range(NQC)]
            )
            st_pos = 0
            st_done_per_c = [0] * NQC

            def st_ready():
                if st_pos >= len(st_units):
                    return False
                c, kc = st_units[st_pos]
                return kt_done[kc] and qt_done[c]

            def st_needs_new_half():
                c, kc = st_units[st_pos]
                need = set()
                for kvp in (2 * kc, 2 * kc + 1):
                    qq = kvp // 4
                    if (c, qq) not in E_tiles:
                        need.add((c, qq))
                return need

            def emit_st():
                nonlocal st_pos
                c, kc = st_units[st_pos]
                st_pos += 1
                for kvp in (2 * kc, 2 * kc + 1):
                    qq, loc = divmod(kvp, 4)
                    E = E_of(c, qq)
                    sp = ps_sp.tile([P, 2 * QCHUNK], F32, tag="sp", name="sp")
                    for h in range(2):
                        kv = 2 * kvp + h
                        nc.tensor.matmul(
                            sp[:, h * QCHUNK:(h + 1) * QCHUNK],
                            kt_t[kv // TPC][:, (kv % TPC) * P:(kv % TPC + 1) * P],
                            qt_t[c],
                            start=True, stop=True,
                        )
                    nc.scalar.activation(
                        out=E[:, 2 * loc:2 * loc + 2, :], in_=sp,
                        func=mybir.ActivationFunctionType.Exp, scale=SCALE,
                    )
                st_done_per_c[c] += 1

            # pv stream: independent units (c, vj) = 4 j-tiles x 4 kv tiles,
            # accumulated into SBUF oacc[c] via DVE adds (no long-lived PSUM
            # chains -> any emission order, E quarters free incrementally).
            pv_all = [(c, vj) for c in range(NQC) for vj in range(NQC)]
            pv_emitted = set()
            pv_done_per_c = [0] * NQC
            quarter_reads = {}   # (c, qq) -> count of consuming pv units emitted

            def pv_candidates():
                for u in pv_all:
                    if u in pv_emitted:
                        continue
                    c, vj = u
                    if st_done_per_c[c] == NQC and vn_done[vj]:
                        yield u

            def pv_ready():
                return next(pv_candidates(), None) is not None

            def pv_remaining():
                return len(pv_all) - len(pv_emitted)

            def pv_blocking_vn():
                for u in pv_all:
                    if u in pv_emitted:
                        continue
                    c, vj = u
                    if st_done_per_c[c] == NQC:
                        return ("v", vj)
                return None

            def emit_pv():
                u = next(pv_candidates(), None)
                assert u is not None
                c, vj = u
                pv_emitted.add(u)
                first = pv_done_per_c[c] == 0
                pv_done_per_c[c] += 1
                for g in range(TPC // 2):      # j-pairs
                    pl = ps_pl.tile([P, 2, VW], F32, tag="pl", name="pl")
                    for j in (2 * g, 2 * g + 1):
                        for kv in range(4 * vj, 4 * vj + 4):
                            E = E_of(c, kv // QKV)
                            nc.tensor.matmul(
                                pl[:, j % 2, 0:AD + 1],
                                E[:, kv % QKV, j * P:(j + 1) * P],
                                vn_t[kv // TPC][:, kv % TPC, 0:AD + 1],
                                start=(kv == 4 * vj), stop=(kv == 4 * vj + 3),
                            )
                    dst = oacc[c][:, 2 * g:2 * g + 2, :]
                    if first:
                        nc.vector.tensor_copy(dst[:, :, 0:AD + 1], pl[:, :, 0:AD + 1])
                    else:
                        nc.vector.tensor_add(
                            dst[:, :, 0:AD + 1], dst[:, :, 0:AD + 1],
                            pl[:, :, 0:AD + 1])
                qq = (c, vj // 2)
                quarter_reads[qq] = quarter_reads.get(qq, 0) + 1
                if quarter_reads[qq] == 2:     # both pv units of this quarter done
                    e_alive.discard(qq)
                    del E_tiles[qq]
                if pv_done_per_c[c] == NQC:    # chunk finished -> normalize + store
                    for j in range(TPC):
                        recip = work.tile([P, 1], F32, tag="recip", name="recip")
                        nc.vector.reciprocal(recip, oacc[c][:, j, AD:AD + 1])
                        o_sb = work.tile([P, AD], F32, tag="o", name="o")
                        nc.vector.tensor_scalar_mul(o_sb, oacc[c][:, j, 0:AD], recip)
                        r0 = c * QCHUNK + j * P
                        nc.sync.dma_start(out=out.ap()[r0:r0 + P, :], in_=o_sb)

            # ---- greedy interleaver ------------------------------------------
            # pace: one spine unit per ~2.67 score units keeps DMA saturated
            while st_pos < len(st_units) or pv_remaining() \
                    or spine_pos < len(spine):
                progressed = False

                # keep DMA ahead of compute (spine leads wall-clock)
                while spine_pos < len(spine) and spine_pos * 5 <= st_pos * 2 + 10:
                    emit_spine()
                    progressed = True

                # scores (up to 1 unit per iteration), with E-slot management
                if st_ready():
                    need = st_needs_new_half()
                    freed = True
                    while len(e_alive) + len([h for h in need if h not in E_tiles]) \
                            > E_SLOTS and freed:
                        if pv_ready():
                            emit_pv()
                            progressed = True
                        else:
                            kind_c = pv_blocking_vn()
                            if kind_c is not None and \
                                    emit_spine_where(lambda u: u == kind_c):
                                progressed = True
                            else:
                                freed = False
                    if len(e_alive) + len([h for h in need if h not in E_tiles]) \
                            <= E_SLOTS:
                        emit_st()
                        progressed = True

                # pv fill: at most one pv unit per score unit (1:1 steady ratio)
                if len(pv_emitted) < st_pos and pv_ready():
                    emit_pv()
                    progressed = True

                if not progressed:
                    # blocked: advance the spine (it unblocks everything)
                    if spine_pos < len(spine):
                        emit_spine()
                    elif pv_ready():
                        emit_pv()
                    elif st_ready():
                        emit_st()
                    else:
                        raise RuntimeError(
                            f"scheduler wedged: st={st_pos} pv={pv_remaining()} "
                            f"spine={spine_pos} alive={len(e_alive)}"
                        )

            assert not E_tiles, E_tiles.keys()

    nc.compile()
    return nc


_NC_CACHE = None


def kernel(q, k, v, Wq, Wk, Wv):
    global _NC_CACHE
    q = np.asarray(q, dtype=np.float32)
    k = np.asarray(k, dtype=np.float32)
    v = np.asarray(v, dtype=np.float32)
    Wq = np.ascontiguousarray(np.asarray(Wq, dtype=np.float32))
    Wk = np.ascontiguousarray(np.asarray(Wk, dtype=np.float32))
    Wv = np.ascontiguousarray(np.asarray(Wv, dtype=np.float32))

    # Shard: batch b -> core b; feature-major layout chosen for the device.
    in_maps = []
    for b in range(B):
        in_maps.append({
            "qT": np.ascontiguousarray(q[b].T),
            "kT": np.ascontiguousarray(k[b].T),
            "vT": np.ascontiguousarray(v[b].T),
            "wq": Wq, "wk": Wk, "wv": Wv,
        })

    if _NC_CACHE is None:
        _NC_CACHE = build_bass()
    nc = _NC_CACHE

    res = run_bass_kernel_spmd(
        nc, in_maps, core_ids=list(range(B)), trace=TRACE, **_TRACE_KW
    )
    if TRACE:
        kernel.last_results = res

    out = np.stack([res.results[b]["out"] for b in range(B)], axis=0)
    return out


# revision 18
# speedup vs baseline: 299.4421x; 299.4421x over previous
"""Cross-attention block kernel for Trainium2 (8 NeuronCores, data-parallel over batch).

Reference computation (per batch element b):
    Q = q[b] @ Wq; K = k[b] @ Wk; V = v[b] @ Wv        # [4096, 128] each
    O = softmax(Q @ K^T / sqrt(128)) @ V               # [4096, 128]

Sharding: one batch element per core (B == n_cores == 8), weights replicated.

Per-core device algorithm (layouts chosen so the PE contracts along
partitions with zero on-device transposes of the big activations):
  - Host supplies q/k/v pre-transposed per core: qT/kT/vT = [1024, 4096]
    (feature-major) - a pure layout transformation done while sharding.
  - Projections (fp32r, moving N=256): QT[AD,q], KT[AD,kv], VT[AD,kv].
  - VT is PE-transposed tile-wise into V natural [kv, AD] (fp16) with an
    appended ones-column (V_aug[kv, 129]) -> the PV matmul then yields both
    the unnormalized output and the softmax denominator in one pass.
  - Scores TRANSPOSED:  ST[kv, q] = lhsT(KT tile).T @ QT chunk   (fp32r)
  - E = exp(ST * scale) on the scalar engine (PSUM->SBUF, fp16 out),
    batched 2 kv-tiles (1024 wide) per activation to amortize the fixed
    ~352-cycle ACTIVATE overhead.  No max-subtraction: scores are O(1)
    for randn-scaled inputs, exp is safe in fp32.  E[kv, q] tiles are
    exactly the stationary operand for the PV matmul.
  - O_aug[q, 129] = sum_kv E_tile.T @ V_aug_tile  (fp16 operands, fp32 PSUM)
    O = O_aug[:, :128] * (1 / O_aug[:, 128]) per row, fp32 out.

Emission is driven by a greedy software-pipeliner that interleaves three
streams under resource bookkeeping so DMA (~134 us of input), PE (~155 us
of matmul) and ACT (~147 us of exp) overlap end to end:
  - spine: per-chunk input DMA + projection units (k/q/v), demand-pulled
    when a downstream stream blocks on them and otherwise paced vs scores;
  - scores: st2(c, kc) units (2 matmuls + 1 exp batched over 2 kv tiles),
    gated on kt/qt and on one of 12 E-quarter SBUF slots;
  - pv: independent units (4 q-tiles x 4 kv tiles) whose PSUM partials are
    folded into per-chunk SBUF accumulators by DVE adds, gated on vn chunks
    and E completeness; pv units are also force-drained whenever scores need
    an E-quarter slot freed (keeps the in-order PE stream deadlock-free
    w.r.t. the sp ping-pong with ACT).
"""

import os
import sys

import numpy as np

for _p in ("/opt/trn_rl_repo",):
    if _p not in sys.path and os.path.isdir(_p):
        sys.path.insert(0, _p)

import concourse.bacc as bacc
import concourse.tile as tile
from concourse import mybir
from concourse.bass_utils import run_bass_kernel_spmd
from concourse.masks import make_identity

F32 = mybir.dt.float32
F32R = mybir.dt.float32r
BF16 = mybir.dt.bfloat16
FP16 = mybir.dt.float16

B, NQ, NKV, QD, KVD, AD = 8, 4096, 4096, 1024, 1024, 128
P = 128
FT = QD // P          # 8 feature tiles (projection contraction tiles)
KVT = NKV // P        # 32 kv tiles
QCHUNK = 512
NQC = NQ // QCHUNK    # 8 q chunks
TPC = QCHUNK // P     # 4 kv/q tiles per chunk
QKV = KVT // 4        # kv tiles per E-quarter (8)
VW = 132              # V_aug tile width (128 vals + ones col + pad)
XH = 256              # input-stream half-chunk width
SCALE = float(AD) ** -0.5
E_SLOTS = 12
OP_SLOTS = 2

TRACE = False         # test.py flips this for profiling runs
_TRACE_KW = {}


def build_bass():
    nc = bacc.Bacc("TRN2", target_bir_lowering=False, debug=False, num_devices=B)

    qT = nc.dram_tensor("qT", [QD, NQ], F32, kind="ExternalInput")
    kT = nc.dram_tensor("kT", [KVD, NKV], F32, kind="ExternalInput")
    vT = nc.dram_tensor("vT", [KVD, NKV], F32, kind="ExternalInput")
    wq = nc.dram_tensor("wq", [QD, AD], F32, kind="ExternalInput")
    wk = nc.dram_tensor("wk", [KVD, AD], F32, kind="ExternalInput")
    wv = nc.dram_tensor("wv", [KVD, AD], F32, kind="ExternalInput")
    out = nc.dram_tensor("out", [NQ, AD], F32, kind="ExternalOutput")

    with tile.TileContext(nc) as tc:
        with (
            tc.tile_pool(name="const", bufs=1) as const,
            tc.tile_pool(name="persist", bufs=1) as persist,
            tc.tile_pool(name="xin", bufs=2) as xin,
            tc.tile_pool(name="work", bufs=2) as work,
            tc.tile_pool(name="epool", bufs=E_SLOTS) as epool,
            tc.tile_pool(name="ps_sp", bufs=3, space="PSUM") as ps_sp,
            tc.tile_pool(name="ps_pl", bufs=2, space="PSUM") as ps_pl,
        ):
            # ---- weights / identity
            w_sb = {}
            for name, w in (("q", wq), ("k", wk), ("v", wv)):
                t = const.tile([P, FT, AD], F32R, tag=f"w{name}", name=f"w{name}")
                nc.sync.dma_start(
                    out=t, in_=w.ap().rearrange("(t p) a -> p t a", p=P).bitcast(F32R)
                )
                w_sb[name] = t
            ident = const.tile([P, P], F32)
            make_identity(nc, ident)

            # ---- per-chunk persistent tiles (separate tags => chunk-granular deps)
            qt_t = [persist.tile([P, QCHUNK], F32R, tag=f"qt{c}", name=f"qt{c}")
                    for c in range(NQC)]
            kt_t = [persist.tile([P, QCHUNK], F32R, tag=f"kt{c}", name=f"kt{c}")
                    for c in range(NQC)]
            vn_t = [persist.tile([P, TPC, VW], FP16, tag=f"vn{c}", name=f"vn{c}")
                    for c in range(NQC)]
            oacc = [persist.tile([P, TPC, VW], F32, tag=f"oacc{c}", name=f"oacc{c}")
                    for c in range(NQC)]

            def proj_half(src_dram, which, c, h):
                """DMA + project one 256-wide half of chunk c; returns psum ap."""
                src = xin.tile([P, FT, XH], F32R, tag="xin", name="xin")
                ap = (src_dram.ap()[:, c * QCHUNK + h * XH: c * QCHUNK + (h + 1) * XH]
                      .rearrange("(t p) n -> p t n", p=P).bitcast(F32R))
                nc.sync.dma_start(out=src, in_=ap)
                pp = ps_sp.tile([P, 2 * QCHUNK], F32, tag="sp", name="pp")
                for t in range(FT):
                    nc.tensor.matmul(
                        pp[:, 0:XH], w_sb[which][:, t, :], src[:, t, :],
                        start=(t == 0), stop=(t == FT - 1),
                    )
                return pp

            def k_chunk(c):
                for h in range(2):
                    pp = proj_half(kT, "k", c, h)
                    nc.vector.tensor_copy(
                        kt_t[c][:, h * XH:(h + 1) * XH], pp[:, 0:XH])

            def q_chunk(c):
                for h in range(2):
                    pp = proj_half(qT, "q", c, h)
                    nc.vector.tensor_copy(
                        qt_t[c][:, h * XH:(h + 1) * XH], pp[:, 0:XH])

            def v_chunk(c):
                vt_sb = work.tile([P, QCHUNK], F32, tag="vt", name="vt")
                for h in range(2):
                    pp = proj_half(vT, "v", c, h)
                    nc.vector.tensor_copy(vt_sb[:, h * XH:(h + 1) * XH], pp[:, 0:XH])
                nc.vector.memset(vn_t[c], 1.0)
                for j in range(TPC):
                    tp = ps_sp.tile([P, 2 * QCHUNK], F32, tag="sp", name="tp")
                    nc.tensor.transpose(tp[:, 0:P], vt_sb[:, j * P:(j + 1) * P], ident)
                    nc.vector.tensor_copy(vn_t[c][:, j, 0:P], tp[:, 0:P])

            # ---- stream state -------------------------------------------------
            spine = [
                ("k", 0), ("q", 0), ("q", 1), ("k", 1), ("q", 2),
                ("k", 2), ("k", 3), ("k", 4), ("k", 5), ("k", 6), ("k", 7),
                ("q", 3), ("v", 0), ("v", 1), ("q", 4), ("v", 2), ("v", 3),
                ("q", 5), ("v", 4), ("v", 5), ("q", 6), ("v", 6), ("v", 7),
                ("q", 7),
            ]
            spine_pos = 0
            kt_done = [False] * NQC
            qt_done = [False] * NQC
            vn_done = [False] * NQC

            def emit_spine():
                nonlocal spine_pos
                kind, c = spine[spine_pos]
                spine_pos += 1
                if kind == "k":
                    k_chunk(c)
                    kt_done[c] = True
                elif kind == "q":
                    q_chunk(c)
                    qt_done[c] = True
                else:
                    v_chunk(c)
                    vn_done[c] = True

            def emit_spine_where(pred):
                while spine_pos < len(spine):
                    if pred(spine[spine_pos]):
                        emit_spine()
                        return True
                    emit_spine()
                return False

            # E-quarter slots
            E_tiles = {}
            e_alive = set()

            def E_of(c, qq):
                key = (c, qq)
                if key not in E_tiles:
                    E_tiles[key] = epool.tile(
                        [P, QKV, QCHUNK], FP16, tag="E", name=f"E{c}_{qq}"
                    )
                    e_alive.add(key)
                return E_tiles[key]

            # scores stream: first 3 chunks ride the k stream, rest c-major
            st_units = (
                [(c, kc) for kc in range(NQC) for c in range(3)]
                + [(c, kc) for c in range(3, NQC) for kc in range(NQC)]
            )
            st_pos = 0
            st_done_per_c = [0] * NQC

            def st_ready():
                if st_pos >= len(st_units):
                    return False
                c, kc = st_units[st_pos]
                return kt_done[kc] and qt_done[c]

            def st_needs_new_half():
                c, kc = st_units[st_pos]
                need = set()
                for kvp in (2 * kc, 2 * kc + 1):
                    qq = kvp // 4
                    if (c, qq) not in E_tiles:
                        need.add((c, qq))
                return need

            def emit_st():
                nonlocal st_pos
                c, kc = st_units[st_pos]
                st_pos += 1
                for kvp in (2 * kc, 2 * kc + 1):
                    qq, loc = divmod(kvp, 4)
                    E = E_of(c, qq)
                    sp = ps_sp.tile([P, 2 * QCHUNK], F32, tag="sp", name="sp")
                    for h in range(2):
                        kv = 2 * kvp + h
                        nc.tensor.matmul(
                            sp[:, h * QCHUNK:(h + 1) * QCHUNK],
                            kt_t[kv // TPC][:, (kv % TPC) * P:(kv % TPC + 1) * P],
                            qt_t[c],
                            start=True, stop=True,
                        )
                    nc.scalar.activation(
                        out=E[:, 2 * loc:2 * loc + 2, :], in_=sp,
                        func=mybir.ActivationFunctionType.Exp, scale=SCALE,
                    )
                st_done_per_c[c] += 1

            # pv stream: independent units (c, vj) = 4 j-tiles x 4 kv tiles,
            # accumulated into SBUF oacc[c] via DVE adds (no long-lived PSUM
            # chains -> any emission order, E quarters free incrementally).
            pv_all = [(c, vj) for c in range(NQC) for vj in range(NQC)]
            pv_emitted = set()
            pv_done_per_c = [0] * NQC
            quarter_reads = {}   # (c, qq) -> count of consuming pv units emitted

            def pv_candidates():
                for u in pv_all:
                    if u in pv_emitted:
                        continue
                    c, vj = u
                    if st_done_per_c[c] == NQC and vn_done[vj]:
                        yield u

            def pv_ready():
                return next(pv_candidates(), None) is not None

            def pv_remaining():
                return len(pv_all) - len(pv_emitted)

            def pv_blocking_vn():
                for u in pv_all:
                    if u in pv_emitted:
                        continue
                    c, vj = u
                    if st_done_per_c[c] == NQC:
                        return ("v", vj)
                return None

            def emit_pv():
                u = next(pv_candidates(), None)
                assert u is not None
                c, vj = u
                pv_emitted.add(u)
                first = pv_done_per_c[c] == 0
                pv_done_per_c[c] += 1
                for g in range(TPC // 2):      # j-pairs
                    pl = ps_pl.tile([P, 2, VW], F32, tag="pl", name="pl")
                    for j in (2 * g, 2 * g + 1):
                        for kv in range(4 * vj, 4 * vj + 4):
                            E = E_of(c, kv // QKV)
                            nc.tensor.matmul(
                                pl[:, j % 2, 0:AD + 1],
                                E[:, kv % QKV, j * P:(j + 1) * P],
                                vn_t[kv // TPC][:, kv % TPC, 0:AD + 1],
                                start=(kv == 4 * vj), stop=(kv == 4 * vj + 3),
                            )
                    dst = oacc[c][:, 2 * g:2 * g + 2, :]
                    if first:
                        nc.vector.tensor_copy(dst[:, :, 0:AD + 1], pl[:, :, 0:AD + 1])
                    else:
                        nc.vector.tensor_add(
                            dst[:, :, 0:AD + 1], dst[:, :, 0:AD + 1],
                            pl[:, :, 0:AD + 1])
                qq = (c, vj // 2)
                quarter_reads[qq] = quarter_reads.get(qq, 0) + 1
                if quarter_reads[qq] == 2:     # both pv units of this quarter done
                    e_alive.discard(qq)
                    del E_tiles[qq]
                if pv_done_per_c[c] == NQC:    # chunk finished -> normalize + store
                    for j in range(TPC):
                        recip = work.tile([P, 1], F32, tag="recip", name="recip")
                        nc.vector.reciprocal(recip, oacc[c][:, j, AD:AD + 1])
                        o_sb = work.tile([P, AD], F32, tag="o", name="o")
                        nc.vector.tensor_scalar_mul(o_sb, oacc[c][:, j, 0:AD], recip)
                        r0 = c * QCHUNK + j * P
                        nc.sync.dma_start(out=out.ap()[r0:r0 + P, :], in_=o_sb)

            # ---- greedy interleaver ------------------------------------------
            # pace: one spine unit per ~2.67 score units keeps DMA saturated
            while st_pos < len(st_units) or pv_remaining() \
                    or spine_pos < len(spine):
                progressed = False

                # keep DMA ahead of compute (spine leads wall-clock)
                while spine_pos < len(spine) and spine_pos * 5 <= st_pos * 2 + 10:
                    emit_spine()
                    progressed = True

                # scores (up to 1 unit per iteration), with E-slot management
                if st_ready():
                    need = st_needs_new_half()
                    freed = True
                    while len(e_alive) + len([h for h in need if h not in E_tiles]) \
                            > E_SLOTS and freed:
                        if pv_ready():
                            emit_pv()
                            progressed = True
                        else:
                            kind_c = pv_blocking_vn()
                            if kind_c is not None and \
                                    emit_spine_where(lambda u: u == kind_c):
                                progressed = True
                            else:
                                freed = False
                    if len(e_alive) + len([h for h in need if h not in E_tiles]) \
                            <= E_SLOTS:
                        emit_st()
                        progressed = True

                # pv fill: at most one pv unit per score unit (1:1 steady ratio)
                if len(pv_emitted) < st_pos and pv_ready():
                    emit_pv()
                    progressed = True

                if not progressed:
                    # blocked: advance the spine (it unblocks everything)
                    if spine_pos < len(spine):
                        emit_spine()
                    elif pv_ready():
                        emit_pv()
                    elif st_ready():
                        emit_st()
                    else:
                        raise RuntimeError(
                            f"scheduler wedged: st={st_pos} pv={pv_remaining()} "
                            f"spine={spine_pos} alive={len(e_alive)}"
                        )

            assert not E_tiles, E_tiles.keys()

    nc.compile()
    return nc


_NC_CACHE = None


def kernel(q, k, v, Wq, Wk, Wv):
    global _NC_CACHE
    q = np.asarray(q, dtype=np.float32)
    k = np.asarray(k, dtype=np.float32)
    v = np.asarray(v, dtype=np.float32)
    Wq = np.ascontiguousarray(np.asarray(Wq, dtype=np.float32))
    Wk = np.ascontiguousarray(np.asarray(Wk, dtype=np.float32))
    Wv = np.ascontiguousarray(np.asarray(Wv, dtype=np.float32))

    # Shard: batch b -> core b; feature-major layout chosen for the device.
    in_maps = []
    for b in range(B):
        in_maps.append({
            "qT": np.ascontiguousarray(q[b].T),
            "kT": np.ascontiguousarray(k[b].T),
            "vT": np.ascontiguousarray(v[b].T),
            "wq": Wq, "wk": Wk, "wv": Wv,
        })

    if _NC_CACHE is None:
        _NC_CACHE = build_bass()
    nc = _NC_CACHE

    res = run_bass_kernel_spmd(
        nc, in_maps, core_ids=list(range(B)), trace=TRACE, **_TRACE_KW
    )
    if TRACE:
        kernel.last_results = res

    out = np.stack([res.results[b]["out"] for b in range(B)], axis=0)
    return out
